# revision 7
# baseline (speedup 1.0000x reference)
"""Trainium2 Bass kernel for nn_AudioSNN: 2-layer spiking NN (snntorch Leaky).

Reference semantics per timestep t (over T=200 steps):
    cur1 = x_t @ w1.T + b1                      # [B, 128]
    m1   = 0.9*m1 + cur1 - (m1_prev > 1)        # reset-by-subtract
    spk1 = (m1 > 1)
    cur2 = spk1 @ w2.T + b2                     # [B, 5]
    m2   = 0.9*m2 + cur2 - (m2_prev > 1)
    out[t] = spk2 = (m2 > 1)

Strategy (pure data-parallel over batch, 8 cores x 1024 batch rows,
software-pipelined, one fused DVE op per step):
  - Transposed layout: states as [feature, batch]; H=128 on SBUF
    partitions, batch on the free dim.
  - One custom DVE op does BOTH membrane updates per step in a single
    instruction over a [128, 1280] state slot: cols 0-1023 hold m1(t),
    cols 1024-1279 hold m2(t-2).  Layer 2 runs two steps late so the
    m1 -> sign -> mm2 -> m2 chain never serializes the DVE recurrence.
    op: out = in0*beta - (in0 > 1) + in1 + s0, in1 = [p1(t) | p2(t-2)]
    from one contiguous PSUM supertile.
  - Spikes via ACT Sign: sg = sign(1 - m1) in {-1,+1}; mm2 uses
    lhsT ~ -0.5*w2.T; the reconstruction constant corr sits in the
    per-partition s0 and is compensated on the m1 side by folding
    (b1 - s0) into mm1 through two constant ones-rows of the K-stack.
  - Matmuls in fp16 with exact hi/lo splits accumulated in fp32 PSUM
    (precision-sim: mm1 needs the 3-term split, mm2 needs hi+lo):
    mm1 = one K=122 pass ([wh;wh;wl;foldh;foldl] . [xh;xl;xh;1;1]),
    two N=512 halves; mm2 = w2h@sg + w2l@sg per 256-wide batch group
    (tile_position col groups).
  - The ones-rows are memset once into persistent x tiles; x DMA writes
    only rows 0-119, in XB-step batches (DMA engine runs ahead).
  - State lives in one [128, 16*1280] ring; every 8 steps the m2-parts
    of 8 ring slots go out with one strided DMA per col group (the
    16-slot ring gives the DMA ~9 iterations of WAR slack); host
    applies the (m2 > 1) threshold.
"""

import numpy as np

import concourse.bacc as bacc
import concourse.mybir as mybir
import concourse.tile as tile
import concourse.dve_ops as dve_ops
from concourse.dve_ops import DveOp
from concourse.dve_spec import Spec, Src0, Src1, C0, C1, C2, lower as dve_lower
from concourse.dve_uop import DveOpSpec
from concourse.bass_utils import run_bass_kernel_spmd

F32 = mybir.dt.float32
F16 = mybir.dt.float16

B, T, F, H, C = 8192, 200, 40, 128, 5
NCORES = 8
BL = B // NCORES          # 1024 batch rows per core
BH = BL // 2              # 512 per mm1 half (PSUM bank limit)
BETA, THR = 0.9, 1.0
NG = 4                    # col-tile groups for layer 2
BG = BL // NG             # 256 batch rows per col group
XB = 4                    # timesteps per x DMA batch
NR = 16                   # state ring slots
OB = 8                    # staging block (steps per out DMA batch)
SW = BL + BG              # state slot width: m1 (1024) + m2 (256)
KS = 3 * F + 2            # mm1 K-stack rows: xh, xl, xh, ones, ones


# --------------------------------------------------------------------------
# Custom DVE op: fused SNN membrane update
# --------------------------------------------------------------------------

def _snn_ref(in0, in1, s0, s1, imm2):
    out = (
        in0.astype(np.float32) * imm2
        - (in0 > s1).astype(np.float32)
        + in1.astype(np.float32)
        + s0
    )
    return out.astype(np.float32)


def _register_snn_op() -> DveOp:
    """out = in0*imm2 - (in0 > s1) + in1 + s0"""
    name = "SNN_MEMBRANE_STEP"
    for op in dve_ops.OPS:
        if op.name == name:
            return op
    body = Src0 * C2 - (Src0 > C1) + Src1 + C0
    spec = Spec(body=body, reference=_snn_ref)
    shas = {}
    for ver in ("v3", "v4"):
        uops = dve_lower(spec, ver=ver)
        shas[ver] = DveOpSpec(name=name, opcode=0, uops=uops, rd1_en=True).sha(ver)
    op = DveOp(name, spec, subdim=False, uops_sha=shas)
    dve_ops.OPS.append(op)
    dve_ops._SUB_OPCODE_FOR_NAME[op.name] = (
        dve_ops._CUSTOM_DVE_ROW_BASE + len(dve_ops.OPS) - 1
    )
    dve_ops.CUSTOM_DVE_SPECS[op.name] = spec
    return op


SNN_OP = _register_snn_op()


# --------------------------------------------------------------------------
# Bass module
# --------------------------------------------------------------------------

def build_module(t_steps: int = T, probe: str = ""):
    assert t_steps % XB == 0
    tb = t_steps // XB
    nb = (t_steps + OB - 1) // OB
    nc = bacc.Bacc("TRN2", target_bir_lowering=False, debug=False)

    XW = XB * BL
    xq = nc.dram_tensor("xq", [tb, 3 * F, XW], F16, kind="ExternalInput").ap()
    w1k = nc.dram_tensor("w1k", [KS, H], F16, kind="ExternalInput").ap()
    w2qh = nc.dram_tensor("w2qh", [H, 32], F16, kind="ExternalInput").ap()
    w2ql = nc.dram_tensor("w2ql", [H, 32], F16, kind="ExternalInput").ap()
    svec = nc.dram_tensor("svec", [128, 1], F32, kind="ExternalInput").ap()
    # out[blk, g, c, j, b]: raw m2 for step tau = OB*blk + j, class c,
    # batch b = g*BG + b
    out = nc.dram_tensor(
        "out", [nb, NG, C, OB, BG], F32, kind="ExternalOutput"
    ).ap()

    with tile.TileContext(nc) as tc:
        with (
            tc.tile_pool(name="const", bufs=1) as cpool,
            tc.tile_pool(name="state", bufs=1) as spool,
            tc.tile_pool(name="ps", bufs=1, space="PSUM") as ppool,
        ):
            w1k_s = cpool.tile([KS, H], F16)
            w2qh_s = cpool.tile([H, 32], F16)
            w2ql_s = cpool.tile([H, 32], F16)
            s0_s = cpool.tile([128, 1], F32)
            nc.sync.dma_start(w1k_s[:], w1k[:])
            nc.sync.dma_start(w2qh_s[:], w2qh[:])
            nc.sync.dma_start(w2ql_s[:], w2ql[:])
            nc.sync.dma_start(s0_s[:], svec[:])

            # state ring: slot i = cols [SW*i, SW*(i+1)); m1 then m2 part
            S = spool.tile([128, NR * SW], F32, tag="S")
            S3 = S[:].rearrange("p (r w) -> p r w", r=NR)
            NX = 4
            xts = [
                spool.tile([KS, XW], F16, tag=f"x{i}", name=f"x{i}")
                for i in range(NX)
            ]
            for xt in xts:
                # base partition must be 32-aligned; rows 96-119 are
                # overwritten by every x DMA, rows 120-121 stay ones
                nc.gpsimd.memset(xt[96:KS, :], 1.0)
                if probe == "no_xdma":
                    nc.gpsimd.memset(xt[0:96, :], 0.01)
            NSG = 3
            sgs = [
                spool.tile([H, BL], F16, tag=f"sg{i}", name=f"sg{i}")
                for i in range(NSG)
            ]
            if probe == "no_act":
                for sg in sgs:
                    nc.gpsimd.memset(sg[:], 1.0)
            NP = 2
            pss = [
                ppool.tile([128, 3 * BH], F32, tag=f"p{i}", name=f"p{i}")
                for i in range(NP)
            ]

            def slot(t):
                return (t - 2) % NR

            # init: slot(-1) zeroed; p2 parts valid before first mm2 (t=2)
            i0 = slot(-1)
            nc.vector.memset(S[:, SW * i0 : SW * (i0 + 1)], 0.0)
            for ps in pss:
                nc.vector.memset(ps[:, BL : BL + BG], 0.0)
                if probe == "no_mm1":
                    nc.vector.memset(ps[:, 0:BL], 0.01)
            if probe == "no_dve":
                nc.vector.memset(S[:], 0.5)

            for t in range(t_steps + 2):
                ps = pss[t % NP]

                # ---- mm1(t): p1 = w1k.T @ [x_t; ones] ----
                if t < t_steps and probe != "no_mm1":
                    k, s = divmod(t, XB)
                    xt = xts[k % NX]
                    if s == 0 and probe != "no_xdma":
                        nc.sync.dma_start(xt[0 : 3 * F, :], xq[k])
                    for half in (0, BH):
                        nc.tensor.matmul(
                            ps[:, half : half + BH],
                            w1k_s[:],
                            xt[:, s * BL + half : s * BL + half + BH],
                            start=True, stop=True,
                        )

                # ---- sign(t-1): sg = sign(1 - m1(t-1)) ----
                if 1 <= t <= t_steps and probe != "no_act":
                    sg = sgs[(t - 1) % NSG]
                    i = slot(t - 1)
                    nc.scalar.activation(
                        sg[:], S[:, SW * i : SW * i + BL],
                        mybir.ActivationFunctionType.Sign,
                        bias=1.0, scale=-1.0,
                    )

                # ---- mm2(t-2): p2 = -0.5*w2 @ sg(t-2), 4 col groups ----
                if 2 <= t <= t_steps + 1 and probe != "no_mm2":
                    sg = sgs[(t - 2) % NSG]
                    for g in range(NG):
                        gs = sg[:, BG * g : BG * (g + 1)]
                        nc.tensor.matmul(
                            ps[32 * g : 32 * (g + 1), BL : BL + BG],
                            w2qh_s[:], gs,
                            start=True, stop=False, tile_position=(0, 32 * g),
                        )
                        nc.tensor.matmul(
                            ps[32 * g : 32 * (g + 1), BL : BL + BG],
                            w2ql_s[:], gs,
                            start=False, stop=True, tile_position=(0, 32 * g),
                        )

                # ---- fused DVE: slot(t) <- [m1(t) | m2(t-2)] ----
                i, ip = slot(t), slot(t - 1)
                if probe != "no_dve":
                    nc.vector._custom_dve(
                        SNN_OP,
                        out=S[:, SW * i : SW * (i + 1)],
                        in0=S[:, SW * ip : SW * (ip + 1)],
                        in1=ps[:, 0:SW],
                        s0=s0_s[:, 0:1], s1=THR, imm2=BETA,
                    )
                    # m2(-1) must be 0 before fused(2) consumes slot(1)
                    if t == 1:
                        nc.vector.memset(
                            S[:, SW * i + BL : SW * (i + 1)], 0.0
                        )

                # ---- staging: m2(tau) sits at slot tau%NR ----
                tau = t - 2
                if tau >= 0 and (tau % OB == OB - 1 or tau == t_steps - 1):
                    if probe != "no_outdma":
                        blk = tau // OB
                        lo = blk * OB
                        n_slots = tau - lo + 1
                        sb = lo % NR
                        for g in range(NG):
                            nc.sync.dma_start(
                                out[blk, g, :, 0:n_slots, :],
                                S3[32 * g : 32 * g + C, sb : sb + n_slots,
                                   BL : BL + BG],
                            )

    nc.compile()
    return nc


_MODULE_CACHE: dict = {}


def _get_module(t_steps: int = T, probe: str = ""):
    key = (t_steps, probe)
    if key not in _MODULE_CACHE:
        _MODULE_CACHE[key] = build_module(t_steps, probe)
    return _MODULE_CACHE[key]


# --------------------------------------------------------------------------
# Host-side sharding / gather
# --------------------------------------------------------------------------

def _fp16_pair(a):
    hi = a.astype(np.float16)
    lo = (a - hi.astype(np.float32)).astype(np.float16)
    return hi, lo


def make_in_maps(x, w1, b1, w2, b2, t_steps: int = T):
    x = np.asarray(x, dtype=np.float32)
    w1 = np.asarray(w1, dtype=np.float32)
    b1 = np.asarray(b1, dtype=np.float32)
    w2 = np.asarray(w2, dtype=np.float32)
    b2 = np.asarray(b2, dtype=np.float32)
    tb = t_steps // XB

    # s0 vector: corr on m2 rows (32g + c), 0 elsewhere
    w2nh, w2nl = _fp16_pair((-0.5 * w2).T)                # [H, C]
    w2qh = np.zeros((H, 32), np.float16)
    w2ql = np.zeros((H, 32), np.float16)
    w2qh[:, :C] = w2nh
    w2ql[:, :C] = w2nl
    w_eff = w2nh.astype(np.float32) + w2nl.astype(np.float32)
    corr = -w_eff.sum(axis=0) + b2                        # [C]
    svec = np.zeros((128, 1), np.float32)
    for g in range(NG):
        svec[32 * g : 32 * g + C, 0] = corr

    # w1 K-stack + (b1 - s0) fold through the ones-rows
    w1h, w1l = _fp16_pair(w1.T)                           # [F, H]
    fold = b1 - svec[:, 0]                                # [H]
    foldh, foldl = _fp16_pair(fold)
    w1k = np.zeros((KS, H), np.float16)
    w1k[0:F] = w1h
    w1k[F : 2 * F] = w1h
    w1k[2 * F : 3 * F] = w1l
    w1k[3 * F] = foldh
    w1k[3 * F + 1] = foldl

    in_maps = []
    for c in range(NCORES):
        xc = x[c * BL : (c + 1) * BL, :t_steps, :]        # [BL, t, F]
        xt_ = xc.transpose(1, 2, 0)                       # [t, F, BL]
        xh16, xl16 = _fp16_pair(xt_)
        trip = np.concatenate([xh16, xl16, xh16], axis=1)  # [t, 120, BL]
        xqc = (
            trip.reshape(tb, XB, 3 * F, BL)
            .transpose(0, 2, 1, 3)
            .reshape(tb, 3 * F, XB * BL)
        )
        in_maps.append(
            {
                "xq": np.ascontiguousarray(xqc),
                "w1k": w1k,
                "w2qh": w2qh,
                "w2ql": w2ql,
                "svec": svec,
            }
        )
    return in_maps


def postprocess(results, t_steps: int = T):
    """results: per-core dicts with 'out' [nb, NG, C, OB, BG] raw m2."""
    outs = []
    for c in range(NCORES):
        r = results[c]["out"]                             # [nb, NG, C, OB, BG]
        nb = r.shape[0]
        spk = (r > THR).astype(np.float32)
        # [nb, NG, C, OB, BG] -> [nb, OB, NG, BG, C] -> [nb*OB, BL, C]
        spk = spk.transpose(0, 3, 1, 4, 2).reshape(nb * OB, BL, C)
        outs.append(spk[:t_steps])
    return np.concatenate(outs, axis=1)                   # [t, B, C]


def kernel(x, w1, b1, w2, b2):
    nc = _get_module(T)
    in_maps = make_in_maps(x, w1, b1, w2, b2, T)
    res = run_bass_kernel_spmd(nc, in_maps, core_ids=list(range(NCORES)))
    return postprocess(res.results, T)
